# revision 2
# baseline (speedup 1.0000x reference)
"""Trainium2 Bass kernel for nn_CartesianPlaneEmbeddingNetwork — matmul-
interpolation architecture (v2).

Math (faithful to the reference, incl. its xz-from-plane_xy quirk):
    p0   = plane_xy[0]                                   (128, 256, 256) f32
    xy   = bilinear(p0, x, y); xz = bilinear(p0, x, z)   per point
    feat = xy * xz * xz
    out  = (sin(30*(feat@W1.T+b1)) -> sin(30*(.@W2.T+b2)) -> @W3.T+b3)

Why matmuls: per-point dma_gather is descriptor-generation-bound on the
single gpsimd engine (~7ns/desc, 2 desc/pt = 904us/core).  Instead the
bilinear sample is computed ON TensorE: for each x-column c and y-half h
the table slab T[c,h] (128 y-rows x 128 channels, fp16) is a matmul
stationary; the moving operand is a host-built sparse column per point
holding its (wy*wx) corner weights at rows y0,y0+1.  PSUM accumulation
over the two x-columns (wx0/wx1-scaled copies) yields xy[c, point]
directly in channel-major layout — no gather, no transpose, ldweights
fully pipelined (measured: back-to-back matmuls sustain 1 col/cycle
@2.4GHz with stationary swaps hidden).

Grouping: points sorted by (x0, ybin3, zbin3); groups = x0 value.
bins: 0 (row0<=126, corners in half0), 1 (row0==127, straddles halves),
2 (>=128, half1).  Per (group, 9-cell) counts are padded to the max over
the 8 cores so one SPMD program fits all cores (zero weight columns in
the padding are harmless).  start=True only on the first matmul touching
each psum range (HW does NOT pre-zero the rest of the bank).

feat = xy*xz^2 on DVE (psum-sourced), MLP over 512-slot windows that
span groups (big ACT activations, 3 matmuls), output DMA'd from a
[1, n] SBUF drain.  gpsimd: completely idle.
"""

import numpy as np

import concourse.bass as bass
import concourse.bacc as bacc
import concourse.mybir as mybir
import concourse.tile as tile

N_CORES = 8
N_TOTAL = 500_000
NPC = N_TOTAL // N_CORES
NG = 255                      # groups: x0 in 0..254
WIN = 512                     # MLP window (psum bank slots)

F32 = mybir.dt.float32
F16 = mybir.dt.float16


# ---------------------------------------------------------------- host math

def point_bins(pts):
    ix = (pts[:, 0] + np.float32(1)) * np.float32(127.5)
    iy = (pts[:, 1] + np.float32(1)) * np.float32(127.5)
    iz = (pts[:, 2] + np.float32(1)) * np.float32(127.5)
    x0 = np.clip(np.floor(ix), 0, 254).astype(np.int32)
    y0 = np.clip(np.floor(iy), 0, 254).astype(np.int32)
    z0 = np.clip(np.floor(iz), 0, 254).astype(np.int32)
    wx1 = (ix - x0).astype(np.float32); wx0 = np.float32(1) - wx1
    wy1 = (iy - y0).astype(np.float32); wy0 = np.float32(1) - wy1
    wz1 = (iz - z0).astype(np.float32); wz0 = np.float32(1) - wz1
    ybin = np.where(y0 <= 126, 0, np.where(y0 == 127, 1, 2)).astype(np.int32)
    zbin = np.where(z0 <= 126, 0, np.where(z0 == 127, 1, 2)).astype(np.int32)
    return dict(x0=x0, y0=y0, z0=z0, wx0=wx0, wx1=wx1, wy0=wy0, wy1=wy1,
                wz0=wz0, wz1=wz1, ybin=ybin, zbin=zbin,
                cell=ybin * 3 + zbin)


def core_counts(b):
    cnt = np.zeros((NG, 9), np.int64)
    np.add.at(cnt, (b["x0"], b["cell"]), 1)
    return cnt


class Layout:
    """Static layout derived from per-(group,cell) caps [NG, 9]."""

    def __init__(self, caps):
        self.caps = caps
        c = caps.reshape(NG, 3, 3)
        self.cell_off = np.zeros((NG, 10), np.int64)
        self.cell_off[:, 1:] = np.cumsum(caps, axis=1)
        self.S_g = self.cell_off[:, 9].copy()
        assert self.S_g.max() <= WIN, self.S_g.max()
        self.slot_off = np.zeros(NG + 1, np.int64)
        self.slot_off[1:] = np.cumsum(self.S_g)
        self.S_total = int(self.slot_off[-1])
        self.n_win = (self.S_total + WIN - 1) // WIN
        self.out_len = self.n_win * WIN

        # M block widths per group.  Blocks (in storage order):
        #   0: y-c0-h0  cells 0..5      1: y-c0-h1  cells 3..8
        #   2: y-c1-h0                  3: y-c1-h1
        #   4: z-c0-h0  runs (y,{0,1})  5: z-c0-h1  runs (y,{1,2})
        #   6: z-c1-h0                  7: z-c1-h1
        y_h0 = c[:, 0:2, :].sum((1, 2)); y_h1 = c[:, 1:3, :].sum((1, 2))
        z_h0 = c[:, :, 0:2].sum((1, 2)); z_h1 = c[:, :, 1:3].sum((1, 2))
        self.bw = np.stack([y_h0, y_h1, y_h0, y_h1,
                            z_h0, z_h1, z_h0, z_h1], axis=1)  # [NG, 8]
        self.boff = np.zeros((NG, 9), np.int64)
        self.boff[:, 1:] = np.cumsum(self.bw, axis=1)
        self.m_g = self.boff[:, 8].copy()
        self.m_off = np.zeros(NG + 1, np.int64)
        self.m_off[1:] = np.cumsum(self.m_g)
        self.M_total = int(self.m_off[-1])
        self.m_gmax = int(self.m_g.max())
        # z-run local offsets within z blocks: run y covers cells (y, za..zb)
        # h0: (y,0)+(y,1)  -> widths c[:, y, 0] + c[:, y, 1]
        # h1: (y,1)+(y,2)
        self.zrun_h0 = np.zeros((NG, 4), np.int64)
        self.zrun_h1 = np.zeros((NG, 4), np.int64)
        for y in range(3):
            self.zrun_h0[:, y + 1] = self.zrun_h0[:, y] + c[:, y, 0] + c[:, y, 1]
            self.zrun_h1[:, y + 1] = self.zrun_h1[:, y] + c[:, y, 1] + c[:, y, 2]

    def key(self):
        return self.caps.tobytes()


def prep_core(pts, lay: Layout):
    """Build this core's M array [128, M_total] f16 and slot map."""
    b = point_bins(pts)
    order = np.lexsort((b["zbin"], b["ybin"], b["x0"]))
    x0 = b["x0"][order]; y0 = b["y0"][order]; z0 = b["z0"][order]
    wx0 = b["wx0"][order]; wx1 = b["wx1"][order]
    wy0 = b["wy0"][order]; wy1 = b["wy1"][order]
    wz0 = b["wz0"][order]; wz1 = b["wz1"][order]
    ybin = b["ybin"][order]; zbin = b["zbin"][order]
    cell = b["cell"][order]

    # index within (group, cell) run
    gc = x0 * 9 + cell
    changes = np.empty(len(gc), bool); changes[0] = True
    changes[1:] = gc[1:] != gc[:-1]
    run_start = np.maximum.accumulate(np.where(changes, np.arange(len(gc)), 0))
    iic = np.arange(len(gc)) - run_start
    slot_local = lay.cell_off[x0, cell] + iic          # within group
    slots = lay.slot_off[x0] + slot_local              # global

    M = np.zeros((128, lay.M_total), np.float16)
    base = lay.m_off[x0]

    def scatter(bt, local_col, rows_vals):
        cols = (base + lay.boff[x0, bt] + local_col)
        for rows, vals, mask in rows_vals:
            if mask is None:
                M[rows, cols] = vals.astype(np.float16)
            else:
                M[rows[mask], cols[mask]] = vals[mask].astype(np.float16)

    # --- y blocks ---
    # h0 block covers cells 0..5 (ybin 0,1): local col = slot_local
    mh0 = ybin <= 1
    lc = slot_local
    for bt, wx in ((0, wx0), (2, wx1)):
        r0 = y0.copy(); r1 = y0 + 1
        v0 = wy0 * wx; v1 = wy1 * wx
        m2 = mh0 & (ybin == 0)          # both corners in h0
        m1 = mh0 & (ybin == 1)          # only row 127 (wy0) in h0
        scatter(bt, lc, [(r0, v0, m2), (r1, v1, m2), (r0, v0, m1)])
    # h1 block covers cells 3..8 (ybin 1,2): local col = slot_local - cell_off[:,3]
    mh1 = ybin >= 1
    lc1 = slot_local - lay.cell_off[x0, 3]
    for bt, wx in ((1, wx0), (3, wx1)):
        r0 = y0 - 128; r1 = y0 - 127
        v0 = wy0 * wx; v1 = wy1 * wx
        m2 = mh1 & (ybin == 2)
        m1 = mh1 & (ybin == 1)          # only row 128 (-> local 0) with wy1
        z0r = np.zeros_like(y0)
        scatter(bt, lc1, [(r0, v0, m2), (r1, v1, m2), (z0r, v1, m1)])
    # --- z blocks ---
    # h0: runs (y, z in {0,1}): local col = zrun_h0[y] + slot_local - cell_off[:, 3y]
    mzh0 = zbin <= 1
    lcz0 = lay.zrun_h0[x0, ybin] + slot_local - lay.cell_off[x0, ybin * 3]
    for bt, wx in ((4, wx0), (6, wx1)):
        r0 = z0.copy(); r1 = z0 + 1
        v0 = wz0 * wx; v1 = wz1 * wx
        m2 = mzh0 & (zbin == 0)
        m1 = mzh0 & (zbin == 1)
        scatter(bt, lcz0, [(r0, v0, m2), (r1, v1, m2), (r0, v0, m1)])
    # h1: runs (y, z in {1,2}): local = zrun_h1[y] + slot_local - cell_off[:, 3y+1]
    mzh1 = zbin >= 1
    lcz1 = lay.zrun_h1[x0, ybin] + slot_local - lay.cell_off[x0, ybin * 3 + 1]
    for bt, wx in ((5, wx0), (7, wx1)):
        r0 = z0 - 128; r1 = z0 - 127
        v0 = wz0 * wx; v1 = wz1 * wx
        m2 = mzh1 & (zbin == 2)
        m1 = mzh1 & (zbin == 1)
        z0r = np.zeros_like(z0)
        scatter(bt, lcz1, [(r0, v0, m2), (r1, v1, m2), (z0r, v1, m1)])

    return {"m": M, "order": order, "slots": slots}


def prep_shared(inputs):
    plane = np.asarray(inputs["plane_xy"], np.float32)[0]        # (C,H,W)
    # tab[:, ((c*2+h)*128 + ch)] with partition k = y-row within half:
    # value = plane[ch, h*128 + k, c]
    t = plane.transpose(2, 1, 0).reshape(256, 2, 128, 128)       # [c, h, k, ch]
    tab = np.ascontiguousarray(t.transpose(2, 0, 1, 3)).reshape(128, -1)
    return {
        "tab": tab.astype(np.float16),
        "w1t": np.ascontiguousarray(np.asarray(inputs["W1"], np.float32).T).astype(np.float16),
        "w2t": np.ascontiguousarray(np.asarray(inputs["W2"], np.float32).T).astype(np.float16),
        "w3t": np.ascontiguousarray(np.asarray(inputs["W3"], np.float32).T).astype(np.float16),
        "b1s": (np.float32(30.0) * np.asarray(inputs["b1"], np.float32)).reshape(128, 1),
        "b2s": (np.float32(30.0) * np.asarray(inputs["b2"], np.float32)).reshape(128, 1),
    }


# ---------------------------------------------------------------- device

def build_nc(lay: Layout, debug_feat=False, debug_interp=False):
    nc = bacc.Bacc("TRN2", target_bir_lowering=False, debug=False,
                   enable_asserts=False, num_devices=N_CORES)
    featdump_d = (nc.dram_tensor("featdump", [128, lay.out_len], F16,
                                 kind="ExternalOutput") if debug_feat else None)
    xydump_d = (nc.dram_tensor("xydump", [128, lay.out_len], F32,
                               kind="ExternalOutput") if debug_interp else None)
    xzdump_d = (nc.dram_tensor("xzdump", [128, lay.out_len], F32,
                               kind="ExternalOutput") if debug_interp else None)

    tab_d = nc.dram_tensor("tab", [128, 256 * 2 * 128], F16, kind="ExternalInput")
    m_d = nc.dram_tensor("m", [128, lay.M_total], F16, kind="ExternalInput")
    w1t_d = nc.dram_tensor("w1t", [128, 128], F16, kind="ExternalInput")
    w2t_d = nc.dram_tensor("w2t", [128, 128], F16, kind="ExternalInput")
    w3t_d = nc.dram_tensor("w3t", [128, 1], F16, kind="ExternalInput")
    b1s_d = nc.dram_tensor("b1s", [128, 1], F32, kind="ExternalInput")
    b2s_d = nc.dram_tensor("b2s", [128, 1], F32, kind="ExternalInput")
    out_d = nc.dram_tensor("out", [lay.out_len], F32, kind="ExternalOutput")

    Sin = mybir.ActivationFunctionType.Sin
    mult = mybir.AluOpType.mult
    TABCH = 16                   # table DMA chunks (16 x 1MB)

    with tile.TileContext(nc) as tc:
        with (
            tc.tile_pool(name="const", bufs=1) as cpool,
            tc.tile_pool(name="mst", bufs=4) as mpool,
            tc.tile_pool(name="ps_i", bufs=2, space="PSUM") as pspool,
            tc.tile_pool(name="xz2", bufs=2) as xpool,
            tc.tile_pool(name="fw", bufs=3) as fpool,
            tc.tile_pool(name="ps_m", bufs=2, space="PSUM") as mlppool,
            tc.tile_pool(name="ps_3", bufs=2, space="PSUM") as p3pool,
            tc.tile_pool(name="hid", bufs=2) as hpool,
            tc.tile_pool(name="ob", bufs=2) as opool,
        ):
            tabs = []
            per = 256 * 2 * 128 // TABCH
            for i in range(TABCH):
                t = cpool.tile([128, per], F16, tag=f"tab{i}")
                nc.sync.dma_start(out=t[:], in_=tab_d.ap()[:, i * per:(i + 1) * per])
                tabs.append(t)

            def tslice(c, h):
                idx = (c * 2 + h) * 128
                return tabs[idx // per][:, idx % per: idx % per + 128]

            def load_const(name, dram, shape, dtype):
                t = cpool.tile(shape, dtype, tag=name)
                nc.sync.dma_start(out=t[:], in_=dram.ap())
                return t

            w1t = load_const("w1t", w1t_d, [128, 128], F16)
            w2t = load_const("w2t", w2t_d, [128, 128], F16)
            w3t = load_const("w3t", w3t_d, [128, 1], F16)
            b1s = load_const("b1s", b1s_d, [128, 1], F32)
            b2s = load_const("b2s", b2s_d, [128, 1], F32)

            co = lay.cell_off; boff = lay.boff
            zr0 = lay.zrun_h0; zr1 = lay.zrun_h1

            # rolling MLP window state
            win = {"tile": None, "idx": 0, "fill": 0}

            def flush_window(n):
                fw = win["tile"]; w = win["idx"]
                if featdump_d is not None:
                    nc.sync.dma_start(
                        out=featdump_d.ap()[:, w * WIN: w * WIN + n],
                        in_=fw[:, :n])
                ps1 = mlppool.tile([128, WIN], F32, tag="ps")
                nc.tensor.matmul(ps1[:, :n], w1t[:], fw[:, :n], start=True, stop=True)
                h1 = hpool.tile([128, WIN], F16, tag="h1")
                nc.scalar.activation(h1[:, :n], ps1[:, :n], Sin, bias=b1s[:], scale=30.0)
                ps2 = mlppool.tile([128, WIN], F32, tag="ps")
                nc.tensor.matmul(ps2[:, :n], w2t[:], h1[:, :n], start=True, stop=True)
                h2 = hpool.tile([128, WIN], F16, tag="h2")
                nc.scalar.activation(h2[:, :n], ps2[:, :n], Sin, bias=b2s[:], scale=30.0)
                ps3 = p3pool.tile([1, WIN], F32, tag="p3")
                nc.tensor.matmul(ps3[:, :n], w3t[:], h2[:, :n], start=True, stop=True)
                ob = opool.tile([1, WIN], F32, tag="ob")
                nc.vector.tensor_scalar_add(out=ob[:, :n], in0=ps3[:, :n],
                                            scalar1=0.0)
                nc.sync.dma_start(out=out_d.ap()[w * WIN: w * WIN + n],
                                  in_=ob[:, :n])
                win["tile"] = None; win["idx"] += 1; win["fill"] = 0

            for g in range(NG):
                S_g = int(lay.S_g[g])
                if S_g == 0:
                    continue
                m_g = int(lay.m_g[g])
                mt = mpool.tile([128, lay.m_gmax], F16, tag="m")
                nc.sync.dma_start(
                    out=mt[:, :m_g],
                    in_=m_d.ap()[:, int(lay.m_off[g]): int(lay.m_off[g]) + m_g])

                xy = pspool.tile([128, WIN], F32, tag="xy")
                xz = pspool.tile([128, WIN], F32, tag="xz")

                # PSUM accumulation-group semantics (measured on HW): the
                # FIRST start=True matmul arms the whole 2KB region — later
                # start=False matmuls read logical zero for bytes not yet
                # written in the group.  A second start=True RE-ARMS the
                # region and discards earlier contributions, so each psum
                # tile gets exactly one start=True (its first matmul).
                armed = {"xy": False, "xz": False}

                def mm(ps, key, c, h, bt, s_a, s_b, b_a):
                    # psum slots [s_a, s_b), M block bt cols [b_a, b_a + n)
                    n = s_b - s_a
                    if n <= 0:
                        return
                    ma = int(boff[g, bt] + b_a)
                    nc.tensor.matmul(ps[:, int(s_a):int(s_b)], tslice(c, h),
                                     mt[:, ma:ma + n], start=not armed[key],
                                     stop=True, skip_group_check=True)
                    armed[key] = True

                c0, c1 = g, g + 1
                # y: h0 block covers slots [co[0], co[6]); h1 block [co[3], co[9])
                mm(xy, "xy", c0, 0, 0, co[g, 0], co[g, 6], 0)
                mm(xy, "xy", c0, 1, 1, co[g, 3], co[g, 9], 0)
                mm(xy, "xy", c1, 0, 2, co[g, 0], co[g, 6], 0)
                mm(xy, "xy", c1, 1, 3, co[g, 3], co[g, 9], 0)
                # z: h0 runs (y, zbin {0,1}); h1 runs (y, zbin {1,2})
                for y in range(3):
                    a, bmid, bend = co[g, 3 * y], co[g, 3 * y + 2], co[g, 3 * y + 3]
                    mm(xz, "xz", c0, 0, 4, a, bmid, zr0[g, y])
                    mm(xz, "xz", c1, 0, 6, a, bmid, zr0[g, y])
                    a1 = co[g, 3 * y + 1]
                    mm(xz, "xz", c0, 1, 5, a1, bend, zr1[g, y])
                    mm(xz, "xz", c1, 1, 7, a1, bend, zr1[g, y])

                s0 = int(lay.slot_off[g])
                if xydump_d is not None:
                    xyd = xpool.tile([128, WIN], F32, tag="xyd", name="xyd")
                    nc.vector.tensor_scalar_add(out=xyd[:, :S_g],
                                                in0=xy[:, :S_g], scalar1=0.0)
                    nc.sync.dma_start(out=xydump_d.ap()[:, s0:s0 + S_g],
                                      in_=xyd[:, :S_g])
                    xzd = xpool.tile([128, WIN], F32, tag="xzd", name="xzd")
                    nc.vector.tensor_scalar_add(out=xzd[:, :S_g],
                                                in0=xz[:, :S_g], scalar1=0.0)
                    nc.sync.dma_start(out=xzdump_d.ap()[:, s0:s0 + S_g],
                                      in_=xzd[:, :S_g])
                xz2 = xpool.tile([128, WIN], F16, tag="xz2")
                nc.scalar.square(xz2[:, :S_g], xz[:, :S_g])

                # feat into rolling 512-windows
                s0 = int(lay.slot_off[g]); pos = 0
                while pos < S_g:
                    if win["tile"] is None:
                        win["tile"] = fpool.tile([128, WIN], F16, tag="fw",
                                                 name="fw")
                    l0 = (s0 + pos) - win["idx"] * WIN
                    take = min(S_g - pos, WIN - l0)
                    nc.vector.tensor_tensor(
                        out=win["tile"][:, l0:l0 + take],
                        in0=xy[:, pos:pos + take],
                        in1=xz2[:, pos:pos + take], op=mult)
                    pos += take
                    win["fill"] = l0 + take
                    if win["fill"] == WIN:
                        flush_window(WIN)
            if win["tile"] is not None:
                flush_window(win["fill"])

    nc.compile()
    return nc


_NC_CACHE = {}


def get_nc(lay: Layout, debug_feat=False, debug_interp=False):
    k = (lay.key(), debug_feat, debug_interp)
    if k not in _NC_CACHE:
        _NC_CACHE[k] = build_nc(lay, debug_feat, debug_interp)
    return _NC_CACHE[k]


LAST_RESULT = None


def kernel(_trace=False, _debug_feat=False, _debug_interp=False, **inputs):
    global LAST_RESULT
    from concourse.bass_utils import run_bass_kernel_spmd

    coords = np.asarray(inputs["coordinates"], np.float32).reshape(-1, 3)
    assert coords.shape[0] == N_TOTAL
    shared = prep_shared(inputs)
    b3 = np.float32(np.asarray(inputs["b3"], np.float32).reshape(-1)[0])

    chunks = [coords[ci * NPC:(ci + 1) * NPC] for ci in range(N_CORES)]
    bins = [point_bins(p) for p in chunks]
    caps = np.stack([core_counts(b) for b in bins]).max(axis=0)
    lay = Layout(caps)
    nc = get_nc(lay, _debug_feat, _debug_interp)

    in_maps, metas = [], []
    for ci in range(N_CORES):
        m = prep_core(chunks[ci], lay)
        metas.append(m)
        in_maps.append({**shared, "m": m["m"]})
    res = run_bass_kernel_spmd(nc, in_maps, core_ids=list(range(N_CORES)),
                               trace=_trace)
    LAST_RESULT = res
    outs = []
    for ci in range(N_CORES):
        r = np.asarray(res.results[ci]["out"], np.float32)
        m = metas[ci]
        full = np.empty(NPC, np.float32)
        full[m["order"]] = r[m["slots"]]
        outs.append(full + b3)
    return np.concatenate(outs).reshape(1, N_TOTAL, 1)
